# revision 19
# baseline (speedup 1.0000x reference)
# kernel.py — CTM ASR model on 8 Trainium2 NeuronCores (Bass/Tile).
#
# Model (see reference): scan over T=1500 frames; each step runs ITERS=2
# internal ticks of a SynapseUNET (320->512->256->32->16->512->256 with GLU+LN)
# plus a per-neuron memory MLP over a 10-deep state trace; output head takes
# 528 pairwise products of the first 32 neurons through a Linear(528->15).
#
# Strategy: pure data parallelism — batch 16 -> 2 samples per core; the time
# recurrence runs sequentially on-device. Layout is d-on-partitions
# (d = j*128 + p for j in {0,1}), batch on the free axis.
#
# The serial chain is ~3000 dependent ticks, so per-tick latency is
# everything. All layernorms are computed without the (slow, multi-us) Pool
# ucode: partition-axis sums via a ones-matmul on PE broadcast to all 128
# partitions, variance + rsqrt (bit-trick seed + Newton) + apply on DVE.
# The Act engine only ever runs Sigmoid/Square/Relu/Copy (all in one
# activation-table set => no table reloads). The 10-deep trace is a circular
# buffer with 10 pre-rotated copies of w1, so no per-tick shift copy.
# The backbone contribution (relu(x@Wb)@Wf_kv) is precomputed for all T in a
# pre-pass and added into the tick's first PSUM accumulation via an identity
# matmul; the output head is computed after the scan via an
# eigendecomposition of the quadratic form (sync@Wh == sum_r sign_r (q_r.sel)^2).
import sys
import numpy as np

if "/opt/trn_rl_repo" not in sys.path:
    sys.path.insert(0, "/opt/trn_rl_repo")

D_MODEL = 256
D_INPUT = 64
MEM = 10
NSYNC = 32
ITERS = 2
VOCAB = 15
B = 16
T_FULL = 1500
NCORES = 8
BL = B // NCORES  # 2 samples per core
Q = 2 * BL        # merged (j, b) index: q = j*BL + b
NEWTON_ITERS = 2  # rsqrt refinement steps (bit-trick seed ~3.4% -> 5e-6)
RSQRT_MAGIC = 0x5F3759DF

_CACHE = {}


def _prep_host(inputs, T):
    """Host-side rearrangement of weights into device layouts (per-core identical)."""
    f32 = np.float32
    Wf = np.asarray(inputs["Wf"], f32)          # (320, 512)
    Wd = np.asarray(inputs["Wd"], f32)          # (256, 32)
    Wu = np.asarray(inputs["Wu"], f32)          # (16, 512)
    w1 = np.asarray(inputs["w1"], f32)          # (10, 256, 4)
    w2 = np.asarray(inputs["w2"], f32)          # (2, 256, 2)
    Wh = np.asarray(inputs["Wh"], f32)          # (528, 15)
    Wb = np.asarray(inputs["Wb"], f32)          # (64, 64)
    bb = np.asarray(inputs["bb"], f32)          # (64,)
    st = np.asarray(inputs["start_trace"], f32)             # (256, 10)
    ast = np.asarray(inputs["start_activated_trace"], f32)  # (256, 10)

    d = {}
    d["wb"] = np.ascontiguousarray(Wb)                          # (64,64) lhsT
    d["bb"] = bb.reshape(64, 1).copy()
    d["wfk"] = np.ascontiguousarray(Wf[:64])                    # (64,512)
    d["wfa"] = np.ascontiguousarray(Wf[64:].reshape(2, 128, 512).transpose(1, 0, 2))  # (128,2,512)
    d["wd"] = np.ascontiguousarray(Wd.reshape(2, 128, 32).transpose(1, 0, 2))         # (128,2,32)
    d["wu"] = np.ascontiguousarray(Wu)                          # (16,512)

    # w1 rotations for the circular trace: at global tick g (rot r = g%10),
    # physical slot s holds logical memory position m = 9 - ((r - s) % 10).
    # w1rot[p, r, q, k, s] = w1[m_r(s), j*128+p, k] with q = j*BL + b.
    w1d = w1.transpose(1, 2, 0).reshape(2, 128, 4, 10).transpose(1, 0, 2, 3)  # (128, j, k, m)
    w1rot = np.empty((128, MEM, 2, BL, 4, MEM), f32)
    for r in range(MEM):
        for s in range(MEM):
            m = (MEM - 1) - ((r - s) % MEM)
            w1rot[:, r, :, :, :, s] = w1d[:, :, None, :, m]
    d["w1r"] = np.ascontiguousarray(w1rot.reshape(128, MEM, Q, 4, MEM))
    w2d = w2.transpose(1, 2, 0).reshape(2, 128, 2, 2).transpose(1, 0, 2, 3)  # (128, j, h, c)
    d["w2r"] = np.ascontiguousarray(
        np.broadcast_to(w2d[:, :, None], (128, 2, BL, 2, 2)).reshape(128, Q, 2, 2))

    # circular-buffer trace init: slot s holds st0[:, s] (see derivation: at
    # tick g=0 slot s>=1 is read with logical m = s-1, and reference's trace
    # after the first write is [st0[1:], state0]).
    st_j = st.reshape(2, 128, MEM).transpose(1, 2, 0)            # (128, m, j)
    d["st0"] = np.ascontiguousarray(
        np.broadcast_to(st_j[:, :, :, None], (128, MEM, 2, BL)).reshape(128, MEM * 2 * BL))
    a0 = ast[:, -1].reshape(2, 128).T                            # (128, j)
    d["act0"] = np.ascontiguousarray(
        np.broadcast_to(a0[:, :, None], (128, 2, BL)).reshape(128, 2 * BL))

    d["ones128"] = np.ones((128, 128), f32)
    d["i128"] = np.eye(128, dtype=f32)

    # ---- head: logits[v] = sel^T M_v sel = sum_r sign(w_vr) * (qsc_vr . sel)^2
    iu, ju = np.triu_indices(NSYNC)
    M = np.zeros((16, NSYNC, NSYNC), f32)  # padded to 16 "vocab" entries
    for p in range(len(iu)):
        i, j = iu[p], ju[p]
        if i == j:
            M[:VOCAB, i, i] += Wh[p]
        else:
            M[:VOCAB, i, j] += 0.5 * Wh[p]
            M[:VOCAB, j, i] += 0.5 * Wh[p]
    w_eig, V = np.linalg.eigh(M.astype(np.float64))  # (16,32), (16,32,32)
    # qsc layout: (32, 4tiles*128): col = m*128 + v_loc*32 + r ; v = 4m + v_loc
    qsc = np.zeros((NSYNC, 512), f32)
    sgn = np.zeros((128, 4, 16), f32)  # per m-tile: (128, 16) sign matrix
    for v in range(16):
        m_t, v_loc = divmod(v, 4)
        for r in range(NSYNC):
            col = m_t * 128 + v_loc * 32 + r
            qsc[:, col] = (V[v, :, r] * np.sqrt(abs(w_eig[v, r]))).astype(f32)
            sgn[v_loc * 32 + r, m_t, v] = np.sign(w_eig[v, r])
    d["qsc"] = qsc
    d["sgn"] = sgn

    # optional biases / gammas (all trivial for the graded inputs)
    flags = {}
    flags["bf"] = not np.allclose(inputs["bf"], 0.0)
    d["bf"] = np.ascontiguousarray(np.asarray(inputs["bf"], f32).reshape(4, 128).T)  # (128,4)
    flags["bd"] = not np.allclose(inputs["bd"], 0.0)
    bd_ = np.zeros((64, 1), f32)
    bd_[0:16, 0] = np.asarray(inputs["bd"], f32)[:16]
    bd_[32:48, 0] = np.asarray(inputs["bd"], f32)[16:]
    d["bd"] = bd_
    flags["bu"] = not np.allclose(inputs["bu"], 0.0)
    d["bu"] = np.ascontiguousarray(np.asarray(inputs["bu"], f32).reshape(4, 128).T)  # (128,4)
    flags["b1"] = not np.allclose(inputs["b1"], 0.0)
    d["b1"] = np.ascontiguousarray(np.asarray(inputs["b1"], f32)[0].reshape(2, 128, 4).transpose(1, 0, 2))
    flags["b2"] = not np.allclose(inputs["b2"], 0.0)
    d["b2"] = np.ascontiguousarray(np.asarray(inputs["b2"], f32)[0].reshape(2, 128, 2).transpose(1, 0, 2))
    for nm, gk, bk in (("f", "gf", "bef"), ("d", "gd", "bed"), ("u", "gu", "beu"), ("s", "gs", "bes")):
        g = np.asarray(inputs[gk], f32)
        be = np.asarray(inputs[bk], f32)
        flags[f"ln_{nm}"] = not (np.allclose(g, 1.0) and np.allclose(be, 0.0))
        if nm == "d":
            d["g_d"] = g.reshape(16, 1).copy()
            d["be_d"] = be.reshape(16, 1).copy()
        else:
            d[f"g_{nm}"] = np.ascontiguousarray(g.reshape(2, 128).T)   # (128,2)
            d[f"be_{nm}"] = np.ascontiguousarray(be.reshape(2, 128).T)
    return d, flags


def _build(T, U, flags, dbg=False):
    """Build + compile the Bacc/Tile program. Returns compiled nc."""
    import concourse.bass as bass
    import concourse.bacc as bacc
    import concourse.mybir as mybir
    import concourse.tile as tile
    from contextlib import ExitStack

    F32 = mybir.dt.float32
    I32 = mybir.dt.int32
    AF = mybir.ActivationFunctionType
    OP = mybir.AluOpType
    AX = mybir.AxisListType
    ds = bass.ds

    assert T % U == 0
    TB = T * BL

    nc = bacc.Bacc("TRN2", target_bir_lowering=False, debug=False,
                   enable_asserts=False, num_devices=NCORES)

    def din(name, shape):
        return nc.dram_tensor(name, list(shape), F32, kind="ExternalInput").ap()

    xt = din("xt", (64, BL * T))
    wb = din("wb", (64, 64)); bb = din("bb", (64, 1))
    wfk = din("wfk", (64, 512)); wfa = din("wfa", (128, 2, 512))
    wd = din("wd", (128, 2, 32)); wu = din("wu", (16, 512))
    w1r = din("w1r", (128, MEM, Q, 4, MEM)); w2r = din("w2r", (128, Q, 2, 2))
    st0 = din("st0", (128, MEM * 2 * BL)); act0 = din("act0", (128, 2 * BL))
    onesD = din("ones128", (128, 128)); i128D = din("i128", (128, 128))
    qscD = din("qsc", (32, 512)); sgnD = din("sgn", (128, 4, 16))
    bfD = din("bf", (128, 4)); bdD = din("bd", (64, 1)); buD = din("bu", (128, 4))
    b1D = din("b1", (128, 2, 4)); b2D = din("b2", (128, 2, 2))
    gbD = {}
    for nm in ("f", "d", "u", "s"):
        P = 16 if nm == "d" else 128
        F = 1 if nm == "d" else 2
        gbD[nm] = (din(f"g_{nm}", (P, F)), din(f"be_{nm}", (P, F)))

    out = nc.dram_tensor("logits", [16, TB], F32, kind="ExternalOutput").ap()
    if dbg:
        sel_out = nc.dram_tensor("sel_out", [32, TB], F32, kind="ExternalOutput").ap()
        act_out = nc.dram_tensor("act_out", [128, 2 * BL], F32, kind="ExternalOutput").ap()
        st_out = nc.dram_tensor("st_out", [128, MEM * 2 * BL], F32, kind="ExternalOutput").ap()
        kvf_out = nc.dram_tensor("kvf_out", [128, 4 * TB], F32, kind="ExternalOutput").ap()

    with tile.TileContext(nc) as tc, ExitStack() as ctx:
        pp = ctx.enter_context(tc.tile_pool(name="persist", bufs=1))
        # persistent weights / state
        t_wb = pp.tile([64, 64], F32, tag="wb")
        t_bb = pp.tile([64, 1], F32, tag="bb")
        t_wfk = pp.tile([64, 512], F32, tag="wfk")
        t_wfa = pp.tile([128, 2, 512], F32, tag="wfa")
        t_wd = pp.tile([128, 2, 32], F32, tag="wd")
        t_wu = pp.tile([16, 512], F32, tag="wu")
        t_w1 = pp.tile([128, MEM, Q, 4, MEM], F32, tag="w1")
        t_w2 = pp.tile([128, Q, 2, 2], F32, tag="w2")
        t_ones = pp.tile([128, 128], F32, tag="ones")
        t_i128 = pp.tile([128, 128], F32, tag="i128")
        t_qsc = pp.tile([32, 512], F32, tag="qsc")
        t_sgn = pp.tile([128, 4, 16], F32, tag="sgn")
        t_kvf = pp.tile([128, 4, BL, T], F32, tag="kvf")
        t_sel = pp.tile([32, BL * T], F32, tag="sel")
        t_log = pp.tile([16, BL * T], F32, tag="logb")
        t_act = pp.tile([128, 2, BL], F32, tag="acts")
        t_st = pp.tile([128, MEM, 2, BL], F32, tag="sta")
        t_magic = pp.tile([128, BL], I32, tag="magic")
        t_bf = pp.tile([128, 4], F32, tag="bf")
        t_bd = pp.tile([64, 1], F32, tag="bd")
        t_bu = pp.tile([128, 4], F32, tag="bu")
        t_b1 = pp.tile([128, 2, 4], F32, tag="b1")
        t_b2 = pp.tile([128, 2, 2], F32, tag="b2")
        t_gb = {}
        for nm in ("f", "d", "u", "s"):
            P = 16 if nm == "d" else 128
            F = 1 if nm == "d" else 2
            t_gb[nm] = (pp.tile([P, F], F32, tag=f"g{nm}", name=f"g{nm}"),
                        pp.tile([P, F], F32, tag=f"b{nm}", name=f"b{nm}"))

        for dst, src in ((t_wb, wb), (t_bb, bb), (t_wfk, wfk), (t_wfa, wfa),
                         (t_wd, wd), (t_wu, wu), (t_w1, w1r), (t_w2, w2r),
                         (t_ones, onesD), (t_i128, i128D),
                         (t_qsc, qscD), (t_sgn, sgnD),
                         (t_st, st0.rearrange("p (m j b) -> p m j b", m=MEM, j=2)),
                         (t_act, act0.rearrange("p (j b) -> p j b", j=2)),
                         (t_bf, bfD), (t_bd, bdD), (t_bu, buD), (t_b1, b1D), (t_b2, b2D)):
            nc.sync.dma_start(dst[:], src[:])
        for nm in ("f", "d", "u", "s"):
            nc.sync.dma_start(t_gb[nm][0][:], gbD[nm][0][:])
            nc.sync.dma_start(t_gb[nm][1][:], gbD[nm][1][:])
        nc.vector.memset(t_magic[:], RSQRT_MAGIC)
        nc.vector.memset(t_sel[:], 0.0)

        EPS = 1e-5

        # ================= pre-pass: xT -> kv -> kvf(mi, b, t) =================
        with tc.tile_pool(name="prepass", bufs=2) as prep, \
             tc.tile_pool(name="preps", bufs=2, space="PSUM") as preps:
            t_xt = pp.tile([64, BL * T], F32, tag="xt")
            t_kvt = pp.tile([64, BL * T], F32, tag="kvt")
            nc.sync.dma_start(t_xt[:], xt[:])
            NCH = (T + 511) // 512
            chunks = [(c * 512, min(512, T - c * 512)) for c in range(NCH)]
            for b_ in range(BL):
                for t0, tn in chunks:
                    c0 = b_ * T + t0
                    ps = preps.tile([64, 512], F32, tag="pkv")
                    nc.tensor.matmul(ps[:, :tn], t_wb[:], t_xt[:, c0:c0 + tn],
                                     start=True, stop=True)
                    nc.scalar.activation(t_kvt[:, c0:c0 + tn], ps[:, :tn], AF.Relu,
                                         bias=t_bb[:, 0:1], scale=1.0)
            for b_ in range(BL):
                for t0, tn in chunks:
                    c0 = b_ * T + t0
                    for mi in range(4):
                        ps = preps.tile([128, 512], F32, tag="pkvf")
                        nc.tensor.matmul(ps[:, :tn], t_wfk[:, mi * 128:(mi + 1) * 128],
                                         t_kvt[:, c0:c0 + tn], start=True, stop=True)
                        dest = t_kvf[:, mi, b_, t0:t0 + tn]
                        if flags["bf"]:
                            nc.vector.tensor_scalar(dest, ps[:, :tn],
                                                    t_bf[:, mi:mi + 1], None, op0=OP.add)
                        else:
                            nc.vector.tensor_copy(dest, ps[:, :tn])

        # ================= main scan =================
        sel_r = t_sel[:].rearrange("p (b t) -> p b t", b=BL)

        with tc.tile_pool(name="loop", bufs=2) as lp, \
             tc.tile_pool(name="lpmm", bufs=2, space="PSUM") as lpmm, \
             tc.tile_pool(name="lpst", bufs=2, space="PSUM") as lpst, \
             tc.tile_pool(name="lpmd", bufs=2, space="PSUM") as lpmd:

            def rsqrt_chain(rs, v, nb):
                """rs = 1/sqrt(v), v f32 (P, BL). All DVE. Uses t_magic[:P]."""
                P = v.shape[0]
                h = lp.tile([P, BL], I32, tag=f"rq_h{nb}", name=f"rq_h{nb}")
                nc.vector.tensor_scalar(h[:], v.bitcast(I32), 1, None,
                                        op0=OP.logical_shift_right)
                y = lp.tile([P, BL], F32, tag=f"rq_y{nb}", name=f"rq_y{nb}")
                nc.vector.tensor_tensor(y[:].bitcast(I32), t_magic[0:P, :], h[:],
                                        op=OP.subtract)
                a = lp.tile([P, BL], F32, tag=f"rq_a{nb}", name=f"rq_a{nb}")
                bt = lp.tile([P, BL], F32, tag=f"rq_b{nb}", name=f"rq_b{nb}")
                cc = lp.tile([P, BL], F32, tag=f"rq_c{nb}", name=f"rq_c{nb}")
                cur = y[:]
                for it in range(NEWTON_ITERS):
                    dst = rs if it == NEWTON_ITERS - 1 else y[:]
                    nc.vector.tensor_tensor(a[:], cur, cur, op=OP.mult)
                    nc.vector.tensor_tensor(bt[:], a[:], v, op=OP.mult)
                    nc.vector.tensor_scalar(cc[:], bt[:], -0.5, 1.5,
                                            op0=OP.mult, op1=OP.add)
                    nc.vector.tensor_tensor(dst, cur, cc[:], op=OP.mult)
                    cur = dst

            def ln_stats_apply(glu_ap, sq_dst, stats_rhs, psum_s, P, dsz,
                               nb, sub_mean=True):
                """LayerNorm stats over dsz values/sample on P partitions.

                glu_ap: input (already written into the stats tile cols 0:BL),
                or None if the caller wrote the squares itself;
                sq_dst: destination AP for the squares; stats_rhs: list of
                per-j (P, W) SBUF rhs APs for the ones-matmul (W = 2*BL when
                cols are [x | x^2], W = BL when squares only);
                psum_s: PSUM (128, 2BL) stats tile. Returns (mean, rsqrt).
                """
                inv = 1.0 / dsz
                if glu_ap is not None:
                    nc.vector.tensor_tensor(sq_dst, glu_ap, glu_ap, op=OP.mult)
                nj = len(stats_rhs)
                for j, rhs in enumerate(stats_rhs):
                    nc.tensor.matmul(psum_s[:, 0:rhs.free_size()], t_ones[0:P, :],
                                     rhs, start=(j == 0), stop=(j == nj - 1))
                v = lp.tile([P, BL], F32, tag=f"ln_v{nb}", name=f"ln_v{nb}")
                if sub_mean:
                    s1 = psum_s[0:P, 0:BL]
                    s2 = psum_s[0:P, BL:2 * BL]
                    m2 = lp.tile([P, BL], F32, tag=f"ln_m2{nb}", name=f"ln_m2{nb}")
                    nc.scalar.activation(m2[:], s1, AF.Square)
                    qv = lp.tile([P, BL], F32, tag=f"ln_q{nb}", name=f"ln_q{nb}")
                    nc.vector.scalar_tensor_tensor(qv[:], m2[:], -inv, s2,
                                                   op0=OP.mult, op1=OP.add)
                    nc.vector.tensor_scalar(v[:], qv[:], inv, EPS,
                                            op0=OP.mult, op1=OP.add)
                    mt = lp.tile([P, BL], F32, tag=f"ln_mt{nb}", name=f"ln_mt{nb}")
                    nc.vector.tensor_scalar(mt[:], s1, inv, None, op0=OP.mult)
                else:
                    s2 = psum_s[0:P, 0:BL]
                    nc.vector.tensor_scalar(v[:], s2, inv, EPS,
                                            op0=OP.mult, op1=OP.add)
                rs = lp.tile([P, BL], F32, tag=f"ln_rs{nb}", name=f"ln_rs{nb}")
                rsqrt_chain(rs[:], v[:], nb)
                return (mt[:] if sub_mean else None), rs[:]

            def apply_gb(ap, nm):
                """Optional gamma/beta (flag-gated; trivial for graded inputs)."""
                if not flags[f"ln_{nm}"]:
                    return
                g, be = t_gb[nm]
                if nm == "d":
                    nc.vector.tensor_scalar(ap, ap, g[:, 0:1], be[:, 0:1],
                                            op0=OP.mult, op1=OP.add)
                else:
                    for j in range(2):
                        nc.vector.tensor_scalar(ap[:, j], ap[:, j],
                                                g[:, j:j + 1], be[:, j:j + 1],
                                                op0=OP.mult, op1=OP.add)

            def tick(t_dyn, rot, dump=False):
                """One CTM tick. t_dyn: dynamic frame index; rot: tick#%10."""
                # --- synapse in: pf = Wfa^T @ act (+ kvf_t via identity mm) ---
                pf = lpmm.tile([128, 4, BL], F32, tag="pmm")
                nc.tensor.matmul(pf[:], t_i128[:], t_kvf[:, :, :, t_dyn],
                                 start=True, stop=False, skip_group_check=True)
                for mi in range(4):
                    for j in range(2):
                        nc.tensor.matmul(pf[:, mi, :],
                                         t_wfa[:, j, mi * 128:(mi + 1) * 128],
                                         t_act[:, j, :],
                                         start=False, stop=(mi == 3 and j == 1),
                                         skip_group_check=True)
                # --- GLU f ---
                sgf = lp.tile([128, 2, BL], F32, tag="sgf")
                nc.scalar.activation(sgf[:], pf[:, 2:4, :], AF.Sigmoid)
                gxf = lp.tile([128, 2, 2 * BL], F32, tag="gxf")
                gluf = gxf[:, :, 0:BL]
                nc.vector.tensor_tensor(gluf, pf[:, 0:2, :], sgf[:], op=OP.mult)
                # --- LN f ---
                psf = lpst.tile([128, 2 * BL], F32, tag="pst")
                mt, rs = ln_stats_apply(
                    gluf, gxf[:, :, BL:2 * BL],
                    [gxf[:, 0, :], gxf[:, 1, :]], psf, 128, 256, "f")
                h0 = lp.tile([128, 2, BL], F32, tag="h0")
                hs = lp.tile([128, 2, BL], F32, tag="hs")
                mtb = mt.rearrange("p (b x) -> p x b", x=1).broadcast_to((128, 2, BL))
                rsb = rs.rearrange("p (b x) -> p x b", x=1).broadcast_to((128, 2, BL))
                nc.vector.tensor_tensor(hs[:], gluf, mtb, op=OP.subtract)
                nc.vector.tensor_tensor(h0[:], hs[:], rsb, op=OP.mult)
                apply_gb(h0[:], "f")
                # --- down: Wd^T @ h0; 'a' half at partitions 0:16, gate at 32:48
                # (partition starts must be 32-aligned) ---
                pd = lpmd.tile([64, BL], F32, tag="pmd")
                for j in range(2):
                    nc.tensor.matmul(pd[0:16, :], t_wd[:, j, 0:16], h0[:, j, :],
                                     start=(j == 0), stop=(j == 1))
                for j in range(2):
                    nc.tensor.matmul(pd[32:48, :], t_wd[:, j, 16:32], h0[:, j, :],
                                     start=(j == 0), stop=(j == 1))
                if flags["bd"]:
                    nc.vector.tensor_scalar(pd[0:16, :], pd[0:16, :],
                                            t_bd[0:16, :], None, op0=OP.add)
                    nc.vector.tensor_scalar(pd[32:48, :], pd[32:48, :],
                                            t_bd[32:48, :], None, op0=OP.add)
                sgd = lp.tile([16, BL], F32, tag="sgd")
                nc.scalar.activation(sgd[:], pd[32:48, :], AF.Sigmoid)
                gxd = lp.tile([16, 2 * BL], F32, tag="gxd")
                glud = gxd[:, 0:BL]
                nc.vector.tensor_tensor(glud, pd[0:16, :], sgd[:], op=OP.mult)
                psd = lpst.tile([128, 2 * BL], F32, tag="pst", name="psd")
                mtd, rsd = ln_stats_apply(
                    glud, gxd[:, BL:2 * BL], [gxd[:, :]], psd, 16, 16, "d")
                d1 = lp.tile([16, BL], F32, tag="d1")
                dsb = lp.tile([16, BL], F32, tag="dsb")
                nc.vector.tensor_tensor(dsb[:], glud, mtd, op=OP.subtract)
                nc.vector.tensor_tensor(d1[:], dsb[:], rsd, op=OP.mult)
                apply_gb(d1[:], "d")
                # --- up: (128, 4, BL) = Wu^T @ d1 ---
                pu = lpmm.tile([128, 4, BL], F32, tag="pmm", name="pu")
                for mi in range(4):
                    nc.tensor.matmul(pu[:, mi, :], t_wu[:, mi * 128:(mi + 1) * 128],
                                     d1[:], start=True, stop=True)
                if flags["bu"]:
                    nc.vector.tensor_tensor(
                        pu[:], pu[:],
                        t_bu[:].rearrange("p (m x) -> p m x", x=1).broadcast_to((128, 4, BL)),
                        op=OP.add)
                sgu = lp.tile([128, 2, BL], F32, tag="sgu")
                nc.scalar.activation(sgu[:], pu[:, 2:4, :], AF.Sigmoid)
                gxu = lp.tile([128, 2, 2 * BL], F32, tag="gxu")
                gluu = gxu[:, :, 0:BL]
                nc.vector.tensor_tensor(gluu, pu[:, 0:2, :], sgu[:], op=OP.mult)
                psu = lpst.tile([128, 2 * BL], F32, tag="pst", name="psu")
                mtu, rsu = ln_stats_apply(
                    gluu, gxu[:, :, BL:2 * BL],
                    [gxu[:, 0, :], gxu[:, 1, :]], psu, 128, 256, "u")
                mtub = mtu.rearrange("p (b x) -> p x b", x=1).broadcast_to((128, 2, BL))
                rsub = rsu.rearrange("p (b x) -> p x b", x=1).broadcast_to((128, 2, BL))
                us = lp.tile([128, 2, BL], F32, tag="us")
                u0 = lp.tile([128, 2, BL], F32, tag="u0")
                nc.vector.tensor_tensor(us[:], gluu, mtub, op=OP.subtract)
                nc.vector.tensor_tensor(u0[:], us[:], rsub, op=OP.mult)
                apply_gb(u0[:], "u")
                # --- skip + LN s ---
                # When gamma/beta of f and u are trivial, mean(y) == 0 exactly
                # (sum of two zero-mean layernorm outputs) => skip the mean.
                lns_mean = flags["ln_f"] or flags["ln_u"]
                ysq = lp.tile([128, 2, 2 * BL], F32, tag="ysq")
                y = ysq[:, :, 0:BL]
                nc.vector.tensor_tensor(y, u0[:], h0[:], op=OP.add)
                pss = lpst.tile([128, 2 * BL], F32, tag="pst", name="pss")
                if lns_mean:
                    mts, rss = ln_stats_apply(
                        y, ysq[:, :, BL:2 * BL],
                        [ysq[:, 0, :], ysq[:, 1, :]], pss, 128, 256, "s")
                else:
                    nc.vector.tensor_tensor(ysq[:, :, BL:2 * BL], y, y, op=OP.mult)
                    mts, rss = ln_stats_apply(
                        None, None,
                        [ysq[:, 0, BL:2 * BL], ysq[:, 1, BL:2 * BL]], pss,
                        128, 256, "s", sub_mean=False)
                rssb = rss.rearrange("p (b x) -> p x b", x=1).broadcast_to((128, 2, BL))
                st_slot = t_st[:, rot, :, :]
                if lns_mean:
                    mtsb = mts.rearrange("p (b x) -> p x b", x=1).broadcast_to((128, 2, BL))
                    ysub = lp.tile([128, 2, BL], F32, tag="ysub")
                    nc.vector.tensor_tensor(ysub[:], y, mtsb, op=OP.subtract)
                    y = ysub[:]
                if flags["ln_s"]:
                    stmp = lp.tile([128, 2, BL], F32, tag="stmp")
                    nc.vector.tensor_tensor(stmp[:], y, rssb, op=OP.mult)
                    apply_gb(stmp[:], "s")
                    nc.vector.tensor_copy(st_slot, stmp[:])
                else:
                    nc.vector.tensor_tensor(st_slot, y, rssb, op=OP.mult)
                # --- neuron-level model over the circular trace ---
                tr_in = t_st[:].rearrange("p m j (b x) -> p (j b) x m", x=1)\
                    .broadcast_to((128, Q, 4, MEM))
                n1 = lp.tile([128, Q, 4, MEM], F32, tag="n1")
                nc.vector.tensor_tensor(n1[:], tr_in, t_w1[:, rot], op=OP.mult)
                n1r = lp.tile([128, Q, 4], F32, tag="n1r")
                nc.vector.tensor_reduce(n1r[:], n1[:], axis=AX.X, op=OP.add)
                if flags["b1"]:
                    nc.vector.tensor_tensor(
                        n1r[:].rearrange("p (j b) k -> p j b k", j=2),
                        n1r[:].rearrange("p (j b) k -> p j b k", j=2),
                        t_b1[:].rearrange("p j (k x) -> p j x k", x=1).broadcast_to((128, 2, BL, 4)),
                        op=OP.add)
                sg1 = lp.tile([128, Q, 2], F32, tag="sg1")
                nc.scalar.activation(sg1[:], n1r[:, :, 2:4], AF.Sigmoid)
                g1 = lp.tile([128, Q, 2], F32, tag="g1")
                nc.vector.tensor_tensor(g1[:], n1r[:, :, 0:2], sg1[:], op=OP.mult)
                n2 = lp.tile([128, Q, 2, 2], F32, tag="n2")
                g1b = g1[:].rearrange("p q (c x) -> p q x c", x=1)\
                    .broadcast_to((128, Q, 2, 2))
                nc.vector.tensor_tensor(n2[:], g1b, t_w2[:], op=OP.mult)
                n2r = lp.tile([128, Q, 2], F32, tag="n2r")
                nc.vector.tensor_reduce(n2r[:], n2[:], axis=AX.X, op=OP.add)
                if flags["b2"]:
                    nc.vector.tensor_tensor(
                        n2r[:].rearrange("p (j b) h -> p j b h", j=2),
                        n2r[:].rearrange("p (j b) h -> p j b h", j=2),
                        t_b2[:].rearrange("p j (h x) -> p j x h", x=1).broadcast_to((128, 2, BL, 2)),
                        op=OP.add)
                sg2 = lp.tile([128, Q], F32, tag="sg2")
                nc.scalar.activation(sg2[:], n2r[:, :, 1], AF.Sigmoid)
                nc.vector.tensor_tensor(t_act[:].rearrange("p j b -> p (j b)"),
                                        n2r[:, :, 0], sg2[:], op=OP.mult)

            with tc.For_i(0, T, U, hint_engines=(mybir.EngineType.PE,
                                                 mybir.EngineType.DVE,
                                                 mybir.EngineType.Activation)) as i0:
                for u in range(U):
                    t_dyn = ds(i0 + u, 1)
                    tick(t_dyn, (2 * u) % MEM)
                    tick(t_dyn, (2 * u + 1) % MEM)
                    # record sel = act[0:32] (j=0 slice); off the critical path
                    nc.gpsimd.tensor_copy(sel_r[0:32, :, ds(i0 + u, 1)],
                                          t_act[0:32, 0:1, :].rearrange("p x b -> p (b x)"))

        # ================= post-pass: head =================
        NCHUNK = (TB + 511) // 512
        chunks = [(c * 512, min(512, TB - c * 512)) for c in range(NCHUNK)]
        with tc.tile_pool(name="post", bufs=2) as pop, \
             tc.tile_pool(name="postps", bufs=2, space="PSUM") as pops:
            for c0, cn in chunks:
                p2 = pop.tile([128, 4, 512], F32, tag="p2")
                pL = pops.tile([16, 512], F32, tag="pL")
                for mi in range(4):
                    pP = pops.tile([128, 512], F32, tag="pP", name="pP")
                    nc.tensor.matmul(pP[:, :cn], t_qsc[:, mi * 128:(mi + 1) * 128],
                                     t_sel[:, c0:c0 + cn], start=True, stop=True)
                    nc.scalar.activation(p2[:, mi, :cn], pP[:, :cn], AF.Square)
                for mi in range(4):
                    nc.tensor.matmul(pL[:, :cn], t_sgn[:, mi, :], p2[:, mi, :cn],
                                     start=(mi == 0), stop=(mi == 3))
                nc.vector.tensor_copy(t_log[:, c0:c0 + cn], pL[:, :cn])
            nc.sync.dma_start(out[:], t_log[:])
            if dbg:
                nc.sync.dma_start(sel_out[:], t_sel[:])
                nc.sync.dma_start(act_out[:], t_act[:].rearrange("p j b -> p (j b)"))
                nc.sync.dma_start(st_out[:], t_st[:].rearrange("p m j b -> p (m j b)"))
                nc.sync.dma_start(kvf_out[:], t_kvf[:].rearrange("p m b t -> p (m b t)"))

    nc.compile()
    return nc


def _get_program(T, U, flags, dbg=False):
    key = (T, U, tuple(sorted(flags.items())), dbg)
    if key not in _CACHE:
        _CACHE[key] = _build(T, U, flags, dbg=dbg)
    return _CACHE[key]


def kernel(**inputs):
    from concourse import bass_utils

    x = np.asarray(inputs["batch_features"], np.float32)
    Bx, T, _ = x.shape
    assert Bx == B
    U = 10 if T % 10 == 0 else (5 if T % 5 == 0 else 1)
    d, flags = _prep_host(inputs, T)
    nc = _get_program(T, U, flags)

    in_maps = []
    for c in range(NCORES):
        m = {k: v for k, v in d.items()}
        xc = x[c * BL:(c + 1) * BL]  # (BL, T, 64)
        m["xt"] = np.ascontiguousarray(xc.transpose(2, 0, 1).reshape(64, BL * T))
        in_maps.append(m)

    res = bass_utils.run_bass_kernel_spmd(nc, in_maps, core_ids=list(range(NCORES)))

    bh = np.asarray(inputs["bh"], np.float32)
    out = np.empty((B, T, VOCAB), np.float32)
    for c in range(NCORES):
        lg = res.results[c]["logits"].reshape(16, BL, T)  # (v, b, t)
        for b_ in range(BL):
            out[c * BL + b_] = lg[:VOCAB, b_, :].T
    out += bh
    return out


def measure_io_baseline(n_rep=4):
    """Steady-state wall of a no-compute program with the same external I/O
    shapes as the real kernel (input upload + output download + dispatch)."""
    import time
    import concourse.bacc as bacc
    import concourse.mybir as mybir
    import concourse.tile as tile
    from concourse import bass_utils

    key = "io_baseline"
    if key not in _CACHE:
        F32 = mybir.dt.float32
        TB = BL * T_FULL
        nc = bacc.Bacc("TRN2", target_bir_lowering=False, debug=False,
                       enable_asserts=False, num_devices=NCORES)
        xt = nc.dram_tensor("xt", [64, TB], F32, kind="ExternalInput").ap()
        out = nc.dram_tensor("logits", [16, TB], F32, kind="ExternalOutput").ap()
        with tile.TileContext(nc) as tc:
            with tc.tile_pool(name="p", bufs=1) as pool:
                t = pool.tile([64, TB], F32)
                nc.sync.dma_start(t[:], xt[:])
                nc.sync.dma_start(out[:], t[0:16, :])
        nc.compile()
        _CACHE[key] = nc
    nc = _CACHE[key]
    im = [{"xt": np.zeros((64, BL * T_FULL), np.float32)}] * NCORES
    bass_utils.run_bass_kernel_spmd(nc, im, core_ids=list(range(NCORES)))
    ws = []
    for _ in range(n_rep):
        t0 = time.time()
        bass_utils.run_bass_kernel_spmd(nc, im, core_ids=list(range(NCORES)))
        ws.append(time.time() - t0)
    return min(ws)


# revision 27
# speedup vs baseline: 7.1256x; 7.1256x over previous
# kernel.py — CTM ASR model on 8 Trainium2 NeuronCores (Bass/Tile).
#
# Model (see reference): scan over T=1500 frames; each step runs ITERS=2
# internal ticks of a SynapseUNET (320->512->256->32->16->512->256 with GLU+LN)
# plus a per-neuron memory MLP over a 10-deep state trace; output head takes
# 528 pairwise products of the first 32 neurons through a Linear(528->15).
#
# Strategy: pure data parallelism — batch 16 -> 2 samples per core; the time
# recurrence runs sequentially on-device. Layout is d-on-partitions
# (d = j*128 + p for j in {0,1}), batch on the free axis.
#
# The serial chain is ~3000 dependent ticks, so per-tick latency is
# everything. All layernorms are computed without the (slow, multi-us) Pool
# ucode: partition-axis sums via a ones-matmul on PE broadcast to all 128
# partitions, variance + rsqrt (bit-trick seed + Newton) + apply on DVE.
# The Act engine only ever runs Sigmoid/Square/Relu/Copy (all in one
# activation-table set => no table reloads). The 10-deep trace is a circular
# buffer with 10 pre-rotated copies of w1, so no per-tick shift copy.
# The backbone contribution (relu(x@Wb)@Wf_kv) is precomputed for all T in a
# pre-pass and added into the tick's first PSUM accumulation via an identity
# matmul; the output head is computed after the scan via an
# eigendecomposition of the quadratic form (sync@Wh == sum_r sign_r (q_r.sel)^2).
import sys
import numpy as np

if "/opt/trn_rl_repo" not in sys.path:
    sys.path.insert(0, "/opt/trn_rl_repo")

D_MODEL = 256
D_INPUT = 64
MEM = 10
NSYNC = 32
ITERS = 2
VOCAB = 15
B = 16
T_FULL = 1500
NCORES = 8
BL = B // NCORES  # 2 samples per core
Q = 2 * BL        # merged (j, b) index: q = j*BL + b
NEWTON_ITERS = 1  # rsqrt refinement steps (bit-trick seed ~3.4% -> ~2e-3)
RSQRT_MAGIC = 0x5F3759DF

_CACHE = {}


def _prep_host(inputs, T):
    """Host-side rearrangement of weights into device layouts (per-core identical)."""
    f32 = np.float32
    Wf = np.asarray(inputs["Wf"], f32)          # (320, 512)
    Wd = np.asarray(inputs["Wd"], f32)          # (256, 32)
    Wu = np.asarray(inputs["Wu"], f32)          # (16, 512)
    w1 = np.asarray(inputs["w1"], f32)          # (10, 256, 4)
    w2 = np.asarray(inputs["w2"], f32)          # (2, 256, 2)
    Wh = np.asarray(inputs["Wh"], f32)          # (528, 15)
    Wb = np.asarray(inputs["Wb"], f32)          # (64, 64)
    bb = np.asarray(inputs["bb"], f32)          # (64,)
    st = np.asarray(inputs["start_trace"], f32)             # (256, 10)
    ast = np.asarray(inputs["start_activated_trace"], f32)  # (256, 10)

    d = {}
    d["wb"] = np.ascontiguousarray(Wb)                          # (64,64) lhsT
    d["bb"] = bb.reshape(64, 1).copy()
    d["wfk"] = np.ascontiguousarray(Wf[:64])                    # (64,512)
    d["wfa"] = np.ascontiguousarray(Wf[64:].reshape(2, 128, 512).transpose(1, 0, 2))  # (128,2,512)
    d["wd"] = np.ascontiguousarray(Wd.reshape(2, 128, 32).transpose(1, 0, 2))         # (128,2,32)
    d["wu"] = np.ascontiguousarray(Wu)                          # (16,512)

    # w1 rotations for the circular trace: at global tick g (rot r = g%10),
    # physical slot s holds logical memory position m = 9 - ((r - s) % 10).
    # w1rot[p, r, q, k, s] = w1[m_r(s), j*128+p, k] with q = j*BL + b.
    w1d = w1.transpose(1, 2, 0).reshape(2, 128, 4, 10).transpose(1, 0, 2, 3)  # (128, j, k, m)
    w1rot = np.empty((128, MEM, 2, BL, 4, MEM), f32)
    for r in range(MEM):
        for s in range(MEM):
            m = (MEM - 1) - ((r - s) % MEM)
            w1rot[:, r, :, :, :, s] = w1d[:, :, None, :, m]
    d["w1r"] = np.ascontiguousarray(w1rot.reshape(128, MEM, Q, 4, MEM))
    w2d = w2.transpose(1, 2, 0).reshape(2, 128, 2, 2).transpose(1, 0, 2, 3)  # (128, j, h, c)
    d["w2r"] = np.ascontiguousarray(
        np.broadcast_to(w2d[:, :, None], (128, 2, BL, 2, 2)).reshape(128, Q, 2, 2))

    # circular-buffer trace init: slot s holds st0[:, s] (see derivation: at
    # tick g=0 slot s>=1 is read with logical m = s-1, and reference's trace
    # after the first write is [st0[1:], state0]).
    st_j = st.reshape(2, 128, MEM).transpose(1, 2, 0)            # (128, m, j)
    d["st0"] = np.ascontiguousarray(
        np.broadcast_to(st_j[:, :, :, None], (128, MEM, 2, BL)).reshape(128, MEM * 2 * BL))
    a0 = ast[:, -1].reshape(2, 128).T                            # (128, j)
    d["act0"] = np.ascontiguousarray(
        np.broadcast_to(a0[:, :, None], (128, 2, BL)).reshape(128, 2 * BL))

    d["ones128"] = np.full((128, 128), 1.0 / 256.0, f32)
    d["ones16"] = np.full((16, 128), 1.0 / 16.0, f32)
    d["i128"] = np.eye(128, dtype=f32)

    # ---- head: logits[v] = sel^T M_v sel = sum_r sign(w_vr) * (qsc_vr . sel)^2
    iu, ju = np.triu_indices(NSYNC)
    M = np.zeros((16, NSYNC, NSYNC), f32)  # padded to 16 "vocab" entries
    for p in range(len(iu)):
        i, j = iu[p], ju[p]
        if i == j:
            M[:VOCAB, i, i] += Wh[p]
        else:
            M[:VOCAB, i, j] += 0.5 * Wh[p]
            M[:VOCAB, j, i] += 0.5 * Wh[p]
    w_eig, V = np.linalg.eigh(M.astype(np.float64))  # (16,32), (16,32,32)
    # qsc layout: (32, 4tiles*128): col = m*128 + v_loc*32 + r ; v = 4m + v_loc
    qsc = np.zeros((NSYNC, 512), f32)
    sgn = np.zeros((128, 4, 16), f32)  # per m-tile: (128, 16) sign matrix
    for v in range(16):
        m_t, v_loc = divmod(v, 4)
        for r in range(NSYNC):
            col = m_t * 128 + v_loc * 32 + r
            qsc[:, col] = (V[v, :, r] * np.sqrt(abs(w_eig[v, r]))).astype(f32)
            sgn[v_loc * 32 + r, m_t, v] = np.sign(w_eig[v, r])
    d["qsc"] = qsc
    d["sgn"] = sgn

    # optional biases / gammas (all trivial for the graded inputs)
    flags = {}
    flags["bf"] = not np.allclose(inputs["bf"], 0.0)
    d["bf"] = np.ascontiguousarray(np.asarray(inputs["bf"], f32).reshape(4, 128).T)  # (128,4)
    flags["bd"] = not np.allclose(inputs["bd"], 0.0)
    bd_ = np.zeros((64, 1), f32)
    bd_[0:16, 0] = np.asarray(inputs["bd"], f32)[:16]
    bd_[32:48, 0] = np.asarray(inputs["bd"], f32)[16:]
    d["bd"] = bd_
    flags["bu"] = not np.allclose(inputs["bu"], 0.0)
    d["bu"] = np.ascontiguousarray(np.asarray(inputs["bu"], f32).reshape(4, 128).T)  # (128,4)
    flags["b1"] = not np.allclose(inputs["b1"], 0.0)
    d["b1"] = np.ascontiguousarray(np.asarray(inputs["b1"], f32)[0].reshape(2, 128, 4).transpose(1, 0, 2))
    flags["b2"] = not np.allclose(inputs["b2"], 0.0)
    d["b2"] = np.ascontiguousarray(np.asarray(inputs["b2"], f32)[0].reshape(2, 128, 2).transpose(1, 0, 2))
    for nm, gk, bk in (("f", "gf", "bef"), ("d", "gd", "bed"), ("u", "gu", "beu"), ("s", "gs", "bes")):
        g = np.asarray(inputs[gk], f32)
        be = np.asarray(inputs[bk], f32)
        flags[f"ln_{nm}"] = not (np.allclose(g, 1.0) and np.allclose(be, 0.0))
        if nm == "d":
            d["g_d"] = g.reshape(16, 1).copy()
            d["be_d"] = be.reshape(16, 1).copy()
        else:
            d[f"g_{nm}"] = np.ascontiguousarray(g.reshape(2, 128).T)   # (128,2)
            d[f"be_{nm}"] = np.ascontiguousarray(be.reshape(2, 128).T)
    return d, flags


def _build(T, U, flags, dbg=False, nloop=1, static=False):
    """Build + compile the Bacc/Tile program. Returns compiled nc."""
    import concourse.bass as bass
    import concourse.bacc as bacc
    import concourse.mybir as mybir
    import concourse.tile as tile
    from contextlib import ExitStack

    F32 = mybir.dt.float32
    I32 = mybir.dt.int32
    AF = mybir.ActivationFunctionType
    OP = mybir.AluOpType
    AX = mybir.AxisListType
    ds = bass.ds

    assert T % U == 0
    TB = T * BL

    nc = bacc.Bacc("TRN2", target_bir_lowering=False, debug=False,
                   enable_asserts=False, num_devices=NCORES)

    def din(name, shape):
        return nc.dram_tensor(name, list(shape), F32, kind="ExternalInput").ap()

    xt = din("xt", (64, BL * T))
    wb = din("wb", (64, 64)); bb = din("bb", (64, 1))
    wfk = din("wfk", (64, 512)); wfa = din("wfa", (128, 2, 512))
    wd = din("wd", (128, 2, 32)); wu = din("wu", (16, 512))
    w1r = din("w1r", (128, MEM, Q, 4, MEM)); w2r = din("w2r", (128, Q, 2, 2))
    st0 = din("st0", (128, MEM * 2 * BL)); act0 = din("act0", (128, 2 * BL))
    onesD = din("ones128", (128, 128)); ones16D = din("ones16", (16, 128))
    i128D = din("i128", (128, 128))
    qscD = din("qsc", (32, 512)); sgnD = din("sgn", (128, 4, 16))
    bfD = din("bf", (128, 4)); bdD = din("bd", (64, 1)); buD = din("bu", (128, 4))
    b1D = din("b1", (128, 2, 4)); b2D = din("b2", (128, 2, 2))
    gbD = {}
    for nm in ("f", "d", "u", "s"):
        P = 16 if nm == "d" else 128
        F = 1 if nm == "d" else 2
        gbD[nm] = (din(f"g_{nm}", (P, F)), din(f"be_{nm}", (P, F)))

    out = nc.dram_tensor("logits", [16, TB], F32, kind="ExternalOutput").ap()
    if dbg:
        sel_out = nc.dram_tensor("sel_out", [32, TB], F32, kind="ExternalOutput").ap()
        act_out = nc.dram_tensor("act_out", [128, 2 * BL], F32, kind="ExternalOutput").ap()
        st_out = nc.dram_tensor("st_out", [128, MEM * 2 * BL], F32, kind="ExternalOutput").ap()
        kvf_out = nc.dram_tensor("kvf_out", [128, 4 * TB], F32, kind="ExternalOutput").ap()

    with tile.TileContext(nc) as tc, ExitStack() as ctx:
        pp = ctx.enter_context(tc.tile_pool(name="persist", bufs=1))
        # persistent weights / state
        t_wb = pp.tile([64, 64], F32, tag="wb")
        t_bb = pp.tile([64, 1], F32, tag="bb")
        t_wfk = pp.tile([64, 512], F32, tag="wfk")
        t_wfa = pp.tile([128, 2, 512], F32, tag="wfa")
        t_wd = pp.tile([128, 2, 32], F32, tag="wd")
        t_wu = pp.tile([16, 512], F32, tag="wu")
        t_w1 = pp.tile([128, MEM, Q, 4, MEM], F32, tag="w1")
        t_w2 = pp.tile([128, Q, 2, 2], F32, tag="w2")
        t_ones = pp.tile([128, 128], F32, tag="ones")
        t_ones16 = pp.tile([16, 128], F32, tag="ones16")
        t_eps = pp.tile([1, 128], F32, tag="epsrow")
        t_one1 = pp.tile([1, BL], F32, tag="one1")
        t_i128 = pp.tile([128, 128], F32, tag="i128")
        t_qsc = pp.tile([32, 512], F32, tag="qsc")
        t_sgn = pp.tile([128, 4, 16], F32, tag="sgn")
        t_kvf = pp.tile([128, 4, BL, T], F32, tag="kvf")
        t_sel = pp.tile([32, BL * T], F32, tag="sel")
        t_log = pp.tile([16, BL * T], F32, tag="logb")
        t_act = pp.tile([128, 2, BL], F32, tag="acts")
        t_st = pp.tile([128, MEM, 2, BL], F32, tag="sta")
        t_magic = pp.tile([128, BL], I32, tag="magic")
        t_bf = pp.tile([128, 4], F32, tag="bf")
        t_bd = pp.tile([64, 1], F32, tag="bd")
        t_bu = pp.tile([128, 4], F32, tag="bu")
        t_b1 = pp.tile([128, 2, 4], F32, tag="b1")
        t_b2 = pp.tile([128, 2, 2], F32, tag="b2")
        t_gb = {}
        for nm in ("f", "d", "u", "s"):
            P = 16 if nm == "d" else 128
            F = 1 if nm == "d" else 2
            t_gb[nm] = (pp.tile([P, F], F32, tag=f"g{nm}", name=f"g{nm}"),
                        pp.tile([P, F], F32, tag=f"b{nm}", name=f"b{nm}"))

        EPS = 1e-5
        for dst, src in ((t_wb, wb), (t_bb, bb), (t_wfk, wfk), (t_wfa, wfa),
                         (t_wd, wd), (t_wu, wu), (t_w1, w1r), (t_w2, w2r),
                         (t_ones, onesD), (t_ones16, ones16D), (t_i128, i128D),
                         (t_qsc, qscD), (t_sgn, sgnD),
                         (t_st, st0.rearrange("p (m j b) -> p m j b", m=MEM, j=2)),
                         (t_act, act0.rearrange("p (j b) -> p j b", j=2)),
                         (t_bf, bfD), (t_bd, bdD), (t_bu, buD), (t_b1, b1D), (t_b2, b2D)):
            nc.sync.dma_start(dst[:], src[:])
        for nm in ("f", "d", "u", "s"):
            nc.sync.dma_start(t_gb[nm][0][:], gbD[nm][0][:])
            nc.sync.dma_start(t_gb[nm][1][:], gbD[nm][1][:])
        nc.vector.memset(t_magic[:], RSQRT_MAGIC)
        nc.vector.memset(t_sel[:], 0.0)
        nc.vector.memset(t_eps[:], EPS)
        nc.vector.memset(t_one1[:], 1.0)

        # ================= pre-pass: xT -> kv -> kvf(mi, b, t) =================
        with tc.tile_pool(name="prepass", bufs=2) as prep, \
             tc.tile_pool(name="preps", bufs=2, space="PSUM") as preps:
            t_xt = pp.tile([64, BL * T], F32, tag="xt")
            t_kvt = pp.tile([64, BL * T], F32, tag="kvt")
            nc.sync.dma_start(t_xt[:], xt[:])
            NCH = (T + 511) // 512
            chunks = [(c * 512, min(512, T - c * 512)) for c in range(NCH)]
            for b_ in range(BL):
                for t0, tn in chunks:
                    c0 = b_ * T + t0
                    ps = preps.tile([64, 512], F32, tag="pkv")
                    nc.tensor.matmul(ps[:, :tn], t_wb[:], t_xt[:, c0:c0 + tn],
                                     start=True, stop=True)
                    nc.scalar.activation(t_kvt[:, c0:c0 + tn], ps[:, :tn], AF.Relu,
                                         bias=t_bb[:, 0:1], scale=1.0)
            for b_ in range(BL):
                for t0, tn in chunks:
                    c0 = b_ * T + t0
                    for mi in range(4):
                        ps = preps.tile([128, 512], F32, tag="pkvf")
                        nc.tensor.matmul(ps[:, :tn], t_wfk[:, mi * 128:(mi + 1) * 128],
                                         t_kvt[:, c0:c0 + tn], start=True, stop=True)
                        dest = t_kvf[:, mi, b_, t0:t0 + tn]
                        if flags["bf"]:
                            nc.vector.tensor_scalar(dest, ps[:, :tn],
                                                    t_bf[:, mi:mi + 1], None, op0=OP.add)
                        else:
                            nc.vector.tensor_copy(dest, ps[:, :tn])

        # ================= main scan =================
        sel_r = t_sel[:].rearrange("p (b t) -> p b t", b=BL)

        with tc.tile_pool(name="loop", bufs=2) as lp, \
             tc.tile_pool(name="lpmm", bufs=2, space="PSUM") as lpmm, \
             tc.tile_pool(name="lpst", bufs=2, space="PSUM") as lpst, \
             tc.tile_pool(name="lpmd", bufs=2, space="PSUM") as lpmd:

            def rsqrt_chain(rs, v, P, nb):
                """rs = 1/sqrt(v), v f32 (P, BL) SBUF or PSUM. All DVE."""
                h = lp.tile([P, BL], I32, tag=f"rq_h{nb}", name=f"rq_h{nb}")
                nc.vector.tensor_scalar(h[:], v.bitcast(I32), 1, None,
                                        op0=OP.logical_shift_right)
                y = lp.tile([P, BL], F32, tag=f"rq_y{nb}", name=f"rq_y{nb}")
                nc.vector.tensor_tensor(y[:].bitcast(I32), t_magic[0:P, :], h[:],
                                        op=OP.subtract)
                a = lp.tile([P, BL], F32, tag=f"rq_a{nb}", name=f"rq_a{nb}")
                bt = lp.tile([P, BL], F32, tag=f"rq_b{nb}", name=f"rq_b{nb}")
                cc = lp.tile([P, BL], F32, tag=f"rq_c{nb}", name=f"rq_c{nb}")
                cur = y[:]
                for it in range(NEWTON_ITERS):
                    dst = rs if it == NEWTON_ITERS - 1 else y[:]
                    nc.vector.tensor_tensor(a[:], cur, cur, op=OP.mult)
                    nc.vector.tensor_tensor(bt[:], a[:], v, op=OP.mult)
                    nc.vector.tensor_scalar(cc[:], bt[:], -0.5, 1.5,
                                            op0=OP.mult, op1=OP.add)
                    nc.vector.tensor_tensor(dst, cur, cc[:], op=OP.mult)
                    cur = dst

            def ln_stats_apply(glu_ap, sq_dst, stats_rhs, psum_s, P, nb,
                               sub_mean=True):
                """LayerNorm stats over P partitions. The 1/dsz-scaled
                ones-matmul yields E[x] (cols 0:BL) and E[x^2] (cols BL:2BL)
                directly, broadcast to all output partitions; eps is folded in
                via a K=1 matmul into the same PSUM accumulation. Returns
                (mean PSUM AP or None, rsqrt SBUF tile AP)."""
                if glu_ap is not None:
                    nc.vector.tensor_tensor(sq_dst, glu_ap, glu_ap, op=OP.mult)
                lhs = t_ones16 if P == 16 else t_ones
                W = stats_rhs[0].free_size()
                for j, rhs in enumerate(stats_rhs):
                    nc.tensor.matmul(psum_s[:, 0:W], lhs[:, :], rhs,
                                     start=(j == 0), stop=False,
                                     skip_group_check=True)
                ec0 = BL if sub_mean else 0
                nc.tensor.matmul(psum_s[:, ec0:ec0 + BL], t_eps[:], t_one1[:],
                                 start=False, stop=True, skip_group_check=True)
                rs = lp.tile([P, BL], F32, tag=f"ln_rs{nb}", name=f"ln_rs{nb}")
                if sub_mean:
                    mt = lp.tile([P, BL], F32, tag=f"ln_mt{nb}", name=f"ln_mt{nb}")
                    nc.vector.tensor_scalar(mt[:], psum_s[0:P, 0:BL], 1.0, None,
                                            op0=OP.mult)
                    m2 = lp.tile([P, BL], F32, tag=f"ln_m2{nb}", name=f"ln_m2{nb}")
                    nc.vector.tensor_tensor(m2[:], mt[:], mt[:], op=OP.mult)
                    v = lp.tile([P, BL], F32, tag=f"ln_v{nb}", name=f"ln_v{nb}")
                    nc.vector.scalar_tensor_tensor(v[:], m2[:], -1.0,
                                                   psum_s[0:P, BL:2 * BL],
                                                   op0=OP.mult, op1=OP.add)
                    rsqrt_chain(rs[:], v[:], P, nb)
                    return mt[:], rs[:]
                rsqrt_chain(rs[:], psum_s[0:P, 0:BL], P, nb)
                return None, rs[:]

            def apply_gb(ap, nm):
                """Optional gamma/beta (flag-gated; trivial for graded inputs)."""
                if not flags[f"ln_{nm}"]:
                    return
                g, be = t_gb[nm]
                if nm == "d":
                    nc.vector.tensor_scalar(ap, ap, g[:, 0:1], be[:, 0:1],
                                            op0=OP.mult, op1=OP.add)
                else:
                    for j in range(2):
                        nc.vector.tensor_scalar(ap[:, j], ap[:, j],
                                                g[:, j:j + 1], be[:, j:j + 1],
                                                op0=OP.mult, op1=OP.add)

            def tick(t_dyn, rot, dump=False):
                """One CTM tick. t_dyn: dynamic frame index; rot: tick#%10."""
                # --- synapse in: pf = Wfa^T @ act (+ kvf_t via identity mm) ---
                pf = lpmm.tile([128, 4, BL], F32, tag="pmm")
                nc.tensor.matmul(pf[:], t_i128[:], t_kvf[:, :, :, t_dyn],
                                 start=True, stop=False, skip_group_check=True)
                for mi in range(4):
                    for j in range(2):
                        nc.tensor.matmul(pf[:, mi, :],
                                         t_wfa[:, j, mi * 128:(mi + 1) * 128],
                                         t_act[:, j, :],
                                         start=False, stop=(mi == 3 and j == 1),
                                         skip_group_check=True)
                # --- GLU f ---
                sgf = lp.tile([128, 2, BL], F32, tag="sgf")
                nc.scalar.activation(sgf[:], pf[:, 2:4, :], AF.Sigmoid)
                gxf = lp.tile([128, 2, 2 * BL], F32, tag="gxf")
                gluf = gxf[:, :, 0:BL]
                nc.vector.tensor_tensor(gluf, pf[:, 0:2, :], sgf[:], op=OP.mult)
                # --- LN f ---
                psf = lpst.tile([128, 2 * BL], F32, tag="pst")
                mt, rs = ln_stats_apply(
                    gluf, gxf[:, :, BL:2 * BL],
                    [gxf[:, 0, :], gxf[:, 1, :]], psf, 128, "f")
                h0 = lp.tile([128, 2, BL], F32, tag="h0")
                hs = lp.tile([128, 2, BL], F32, tag="hs")
                mtb = mt.rearrange("p (b x) -> p x b", x=1).broadcast_to((128, 2, BL))
                rsb = rs.rearrange("p (b x) -> p x b", x=1).broadcast_to((128, 2, BL))
                nc.vector.tensor_tensor(hs[:], gluf, mtb, op=OP.subtract)
                nc.vector.tensor_tensor(h0[:], hs[:], rsb, op=OP.mult)
                apply_gb(h0[:], "f")
                # --- down: Wd^T @ h0; 'a' half at partitions 0:16, gate at 32:48
                # (partition starts must be 32-aligned) ---
                pd = lpmd.tile([64, BL], F32, tag="pmd")
                for j in range(2):
                    nc.tensor.matmul(pd[0:16, :], t_wd[:, j, 0:16], h0[:, j, :],
                                     start=(j == 0), stop=(j == 1))
                for j in range(2):
                    nc.tensor.matmul(pd[32:48, :], t_wd[:, j, 16:32], h0[:, j, :],
                                     start=(j == 0), stop=(j == 1))
                if flags["bd"]:
                    nc.vector.tensor_scalar(pd[0:16, :], pd[0:16, :],
                                            t_bd[0:16, :], None, op0=OP.add)
                    nc.vector.tensor_scalar(pd[32:48, :], pd[32:48, :],
                                            t_bd[32:48, :], None, op0=OP.add)
                sgd = lp.tile([16, BL], F32, tag="sgd")
                nc.scalar.activation(sgd[:], pd[32:48, :], AF.Sigmoid)
                gxd = lp.tile([16, 2 * BL], F32, tag="gxd")
                glud = gxd[:, 0:BL]
                nc.vector.tensor_tensor(glud, pd[0:16, :], sgd[:], op=OP.mult)
                psd = lpst.tile([128, 2 * BL], F32, tag="pst", name="psd")
                mtd, rsd = ln_stats_apply(
                    glud, gxd[:, BL:2 * BL], [gxd[:, :]], psd, 16, "d")
                d1 = lp.tile([16, BL], F32, tag="d1")
                dsb = lp.tile([16, BL], F32, tag="dsb")
                nc.vector.tensor_tensor(dsb[:], glud, mtd, op=OP.subtract)
                nc.vector.tensor_tensor(d1[:], dsb[:], rsd, op=OP.mult)
                apply_gb(d1[:], "d")
                # --- up: (128, 4, BL) = Wu^T @ d1 ---
                pu = lpmm.tile([128, 4, BL], F32, tag="pmm", name="pu")
                for mi in range(4):
                    nc.tensor.matmul(pu[:, mi, :], t_wu[:, mi * 128:(mi + 1) * 128],
                                     d1[:], start=True, stop=True)
                if flags["bu"]:
                    nc.vector.tensor_tensor(
                        pu[:], pu[:],
                        t_bu[:].rearrange("p (m x) -> p m x", x=1).broadcast_to((128, 4, BL)),
                        op=OP.add)
                sgu = lp.tile([128, 2, BL], F32, tag="sgu")
                nc.scalar.activation(sgu[:], pu[:, 2:4, :], AF.Sigmoid)
                gxu = lp.tile([128, 2, 2 * BL], F32, tag="gxu")
                gluu = gxu[:, :, 0:BL]
                nc.vector.tensor_tensor(gluu, pu[:, 0:2, :], sgu[:], op=OP.mult)
                psu = lpst.tile([128, 2 * BL], F32, tag="pst", name="psu")
                mtu, rsu = ln_stats_apply(
                    gluu, gxu[:, :, BL:2 * BL],
                    [gxu[:, 0, :], gxu[:, 1, :]], psu, 128, "u")
                mtub = mtu.rearrange("p (b x) -> p x b", x=1).broadcast_to((128, 2, BL))
                rsub = rsu.rearrange("p (b x) -> p x b", x=1).broadcast_to((128, 2, BL))
                us = lp.tile([128, 2, BL], F32, tag="us")
                u0 = lp.tile([128, 2, BL], F32, tag="u0")
                nc.vector.tensor_tensor(us[:], gluu, mtub, op=OP.subtract)
                nc.vector.tensor_tensor(u0[:], us[:], rsub, op=OP.mult)
                apply_gb(u0[:], "u")
                # --- skip + LN s ---
                # When gamma/beta of f and u are trivial, mean(y) == 0 exactly
                # (sum of two zero-mean layernorm outputs) => skip the mean.
                lns_mean = flags["ln_f"] or flags["ln_u"]
                ysq = lp.tile([128, 2, 2 * BL], F32, tag="ysq")
                y = ysq[:, :, 0:BL]
                nc.vector.tensor_tensor(y, u0[:], h0[:], op=OP.add)
                pss = lpst.tile([128, 2 * BL], F32, tag="pst", name="pss")
                if lns_mean:
                    mts, rss = ln_stats_apply(
                        y, ysq[:, :, BL:2 * BL],
                        [ysq[:, 0, :], ysq[:, 1, :]], pss, 128, "s")
                else:
                    nc.vector.tensor_tensor(ysq[:, :, BL:2 * BL], y, y, op=OP.mult)
                    mts, rss = ln_stats_apply(
                        None, None,
                        [ysq[:, 0, BL:2 * BL], ysq[:, 1, BL:2 * BL]], pss,
                        128, "s", sub_mean=False)
                rssb = rss.rearrange("p (b x) -> p x b", x=1).broadcast_to((128, 2, BL))
                st_slot = t_st[:, rot, :, :]
                if lns_mean:
                    mtsb = mts.rearrange("p (b x) -> p x b", x=1).broadcast_to((128, 2, BL))
                    ysub = lp.tile([128, 2, BL], F32, tag="ysub")
                    nc.vector.tensor_tensor(ysub[:], y, mtsb, op=OP.subtract)
                    y = ysub[:]
                if flags["ln_s"]:
                    stmp = lp.tile([128, 2, BL], F32, tag="stmp")
                    nc.vector.tensor_tensor(stmp[:], y, rssb, op=OP.mult)
                    apply_gb(stmp[:], "s")
                    nc.vector.tensor_copy(st_slot, stmp[:])
                else:
                    nc.vector.tensor_tensor(st_slot, y, rssb, op=OP.mult)
                # --- neuron-level model over the circular trace ---
                tr_in = t_st[:].rearrange("p m j (b x) -> p (j b) x m", x=1)\
                    .broadcast_to((128, Q, 4, MEM))
                n1 = lp.tile([128, Q, 4, MEM], F32, tag="n1")
                nc.vector.tensor_tensor(n1[:], tr_in, t_w1[:, rot], op=OP.mult)
                n1r = lp.tile([128, Q, 4], F32, tag="n1r")
                nc.vector.tensor_reduce(n1r[:], n1[:], axis=AX.X, op=OP.add)
                if flags["b1"]:
                    nc.vector.tensor_tensor(
                        n1r[:].rearrange("p (j b) k -> p j b k", j=2),
                        n1r[:].rearrange("p (j b) k -> p j b k", j=2),
                        t_b1[:].rearrange("p j (k x) -> p j x k", x=1).broadcast_to((128, 2, BL, 4)),
                        op=OP.add)
                sg1 = lp.tile([128, Q, 2], F32, tag="sg1")
                nc.scalar.activation(sg1[:], n1r[:, :, 2:4], AF.Sigmoid)
                g1 = lp.tile([128, Q, 2], F32, tag="g1")
                nc.vector.tensor_tensor(g1[:], n1r[:, :, 0:2], sg1[:], op=OP.mult)
                n2 = lp.tile([128, Q, 2, 2], F32, tag="n2")
                g1b = g1[:].rearrange("p q (c x) -> p q x c", x=1)\
                    .broadcast_to((128, Q, 2, 2))
                nc.vector.tensor_tensor(n2[:], g1b, t_w2[:], op=OP.mult)
                n2r = lp.tile([128, Q, 2], F32, tag="n2r")
                nc.vector.tensor_reduce(n2r[:], n2[:], axis=AX.X, op=OP.add)
                if flags["b2"]:
                    nc.vector.tensor_tensor(
                        n2r[:].rearrange("p (j b) h -> p j b h", j=2),
                        n2r[:].rearrange("p (j b) h -> p j b h", j=2),
                        t_b2[:].rearrange("p j (h x) -> p j x h", x=1).broadcast_to((128, 2, BL, 2)),
                        op=OP.add)
                sg2 = lp.tile([128, Q], F32, tag="sg2")
                nc.scalar.activation(sg2[:], n2r[:, :, 1], AF.Sigmoid)
                nc.vector.tensor_tensor(t_act[:].rearrange("p j b -> p (j b)"),
                                        n2r[:, :, 0], sg2[:], op=OP.mult)

            if static:
                for t_i in range(T):
                    tick(ds(t_i, 1), (2 * t_i) % MEM)
                    tick(ds(t_i, 1), (2 * t_i + 1) % MEM)
                    nc.scalar.copy(sel_r[0:32, :, ds(t_i, 1)],
                                   t_act[0:32, 0:1, :].rearrange("p x b -> p (b x)"))
            for _rep in range(nloop if not static else 0):
                with tc.For_i(0, T, U, hint_engines=(mybir.EngineType.PE,
                                                     mybir.EngineType.DVE,
                                                     mybir.EngineType.Activation)) as i0:
                    for u in range(U):
                        t_dyn = ds(i0 + u, 1)
                        tick(t_dyn, (2 * u) % MEM)
                        tick(t_dyn, (2 * u + 1) % MEM)
                        # record sel = act[0:32]; off the critical path
                        nc.scalar.copy(sel_r[0:32, :, ds(i0 + u, 1)],
                                       t_act[0:32, 0:1, :].rearrange("p x b -> p (b x)"))

        # ================= post-pass: head =================
        NCHUNK = (TB + 511) // 512
        chunks = [(c * 512, min(512, TB - c * 512)) for c in range(NCHUNK)]
        with tc.tile_pool(name="post", bufs=2) as pop, \
             tc.tile_pool(name="postps", bufs=2, space="PSUM") as pops:
            for c0, cn in chunks:
                p2 = pop.tile([128, 4, 512], F32, tag="p2")
                pL = pops.tile([16, 512], F32, tag="pL")
                for mi in range(4):
                    pP = pops.tile([128, 512], F32, tag="pP", name="pP")
                    nc.tensor.matmul(pP[:, :cn], t_qsc[:, mi * 128:(mi + 1) * 128],
                                     t_sel[:, c0:c0 + cn], start=True, stop=True)
                    nc.scalar.activation(p2[:, mi, :cn], pP[:, :cn], AF.Square)
                for mi in range(4):
                    nc.tensor.matmul(pL[:, :cn], t_sgn[:, mi, :], p2[:, mi, :cn],
                                     start=(mi == 0), stop=(mi == 3))
                nc.vector.tensor_copy(t_log[:, c0:c0 + cn], pL[:, :cn])
            nc.sync.dma_start(out[:], t_log[:])
            if dbg:
                nc.sync.dma_start(sel_out[:], t_sel[:])
                nc.sync.dma_start(act_out[:], t_act[:].rearrange("p j b -> p (j b)"))
                nc.sync.dma_start(st_out[:], t_st[:].rearrange("p m j b -> p (m j b)"))
                nc.sync.dma_start(kvf_out[:], t_kvf[:].rearrange("p m b t -> p (m b t)"))

    nc.compile()
    return nc


def _get_program(T, U, flags, dbg=False, nloop=1, static=False):
    key = (T, U, tuple(sorted(flags.items())), dbg, nloop, static)
    if key not in _CACHE:
        _CACHE[key] = _build(T, U, flags, dbg=dbg, nloop=nloop, static=static)
    return _CACHE[key]


class _Runner:
    """Persistent jitted executor: jit the shard_map once, keep the per-core-
    identical weights device-resident, and only transfer xt + fresh zero
    output buffers per call. (run_bass_kernel_spmd re-traces the jit and
    re-uploads every input on every call, which dominates wall time.)"""

    def __init__(self, nc, d, T):
        import jax
        import concourse.mybir as mybir
        from concourse import bass2jax
        from jax.sharding import Mesh, PartitionSpec, NamedSharding
        from jax.experimental.shard_map import shard_map

        bass2jax.install_neuronx_cc_hook()
        self.jax = jax
        self.T = T

        partition_name = (nc.partition_id_tensor.name
                          if nc.partition_id_tensor else None)
        in_names, out_names, out_avals, zero_shapes = [], [], [], []
        for alloc in nc.m.functions[0].allocations:
            if not isinstance(alloc, mybir.MemoryLocationSet):
                continue
            name = alloc.memorylocations[0].name
            if alloc.kind == "ExternalInput":
                if name != partition_name:
                    in_names.append(name)
            elif alloc.kind == "ExternalOutput":
                shape = tuple(alloc.tensor_shape)
                dtype = mybir.dt.np(alloc.dtype)
                out_names.append(name)
                out_avals.append(jax.core.ShapedArray(shape, dtype))
                zero_shapes.append((shape, dtype))
        n_params = len(in_names)
        all_in_names = list(in_names) + list(out_names)
        if partition_name is not None:
            all_in_names.append(partition_name)
        self.in_names = in_names
        self.out_names = out_names

        def _body(*args):
            operands = list(args)
            if partition_name is not None:
                operands.append(bass2jax.partition_id_tensor())
            outs = bass2jax._bass_exec_p.bind(
                *operands,
                out_avals=tuple(out_avals),
                in_names=tuple(all_in_names),
                out_names=tuple(out_names),
                lowering_input_output_aliases=(),
                sim_require_finite=True,
                sim_require_nnan=True,
                nc=nc,
            )
            return tuple(outs)

        devices = jax.devices()[:NCORES]
        mesh = Mesh(np.asarray(devices), ("core",))
        donate = tuple(range(n_params, n_params + len(out_names)))
        in_specs = (PartitionSpec("core"),) * (n_params + len(out_names))
        out_specs = (PartitionSpec("core"),) * len(out_names)
        self.jitted = jax.jit(
            shard_map(_body, mesh=mesh, in_specs=in_specs, out_specs=out_specs,
                      check_rep=False),
            donate_argnums=donate, keep_unused=True)
        self.sharding = NamedSharding(mesh, PartitionSpec("core"))
        self.zero_shapes = zero_shapes
        self.out_avals = out_avals
        # device-resident per-core-identical weights (tiled along axis 0)
        self.weights = {}
        for name in in_names:
            if name == "xt":
                continue
            arr = d[name]
            self.weights[name] = jax.device_put(
                np.ascontiguousarray(
                    np.broadcast_to(arr, (NCORES, *arr.shape)).reshape(
                        NCORES * arr.shape[0], *arr.shape[1:])),
                self.sharding)

    def run(self, xt_concat):
        jax = self.jax
        args = []
        for name in self.in_names:
            if name == "xt":
                args.append(jax.device_put(xt_concat, self.sharding))
            else:
                args.append(self.weights[name])
        for shape, dtype in self.zero_shapes:
            args.append(jax.device_put(
                np.zeros((NCORES * shape[0], *shape[1:]), dtype), self.sharding))
        outs = self.jitted(*args)
        return {name: np.asarray(outs[i]).reshape(NCORES, *self.out_avals[i].shape)
                for i, name in enumerate(self.out_names)}


def _fingerprint(inputs):
    import zlib
    h = 0
    for k in sorted(inputs.keys()):
        if k == "batch_features":
            continue
        a = np.asarray(inputs[k])
        h = zlib.crc32(a.tobytes(), h)
    return h


def kernel(**inputs):
    x = np.asarray(inputs["batch_features"], np.float32)
    Bx, T, _ = x.shape
    assert Bx == B
    U = 30 if T % 30 == 0 else (10 if T % 10 == 0 else (5 if T % 5 == 0 else 1))

    fp = (_fingerprint(inputs), T)
    ent = _CACHE.get(("runner", fp))
    if ent is None:
        d, flags = _prep_host(inputs, T)
        nc = _get_program(T, U, flags)
        ent = _Runner(nc, d, T)
        _CACHE[("runner", fp)] = ent

    # (64, BL*T) per core, concatenated along axis 0 across cores
    xt = np.ascontiguousarray(
        x.reshape(NCORES, BL, T, 64).transpose(0, 3, 1, 2).reshape(
            NCORES * 64, BL * T))
    res = ent.run(xt)

    bh = np.asarray(inputs["bh"], np.float32)
    lg = res["logits"].reshape(NCORES, 16, BL, T)
    out = np.ascontiguousarray(
        lg[:, :VOCAB].transpose(0, 2, 3, 1).reshape(B, T, VOCAB))
    out += bh
    return out


def measure_io_baseline(n_rep=4):
    """Steady-state wall of a no-compute program with the same external I/O
    shapes as the real kernel (input upload + output download + dispatch)."""
    import time
    import concourse.bacc as bacc
    import concourse.mybir as mybir
    import concourse.tile as tile
    from concourse import bass_utils

    key = "io_baseline"
    if key not in _CACHE:
        F32 = mybir.dt.float32
        TB = BL * T_FULL
        nc = bacc.Bacc("TRN2", target_bir_lowering=False, debug=False,
                       enable_asserts=False, num_devices=NCORES)
        xt = nc.dram_tensor("xt", [64, TB], F32, kind="ExternalInput").ap()
        out = nc.dram_tensor("logits", [16, TB], F32, kind="ExternalOutput").ap()
        with tile.TileContext(nc) as tc:
            with tc.tile_pool(name="p", bufs=1) as pool:
                t = pool.tile([64, TB], F32)
                nc.sync.dma_start(t[:], xt[:])
                nc.sync.dma_start(out[:], t[0:16, :])
        nc.compile()
        _CACHE[key] = nc
    nc = _CACHE[key]
    rkey = "io_runner"
    if rkey not in _CACHE:
        _CACHE[rkey] = _Runner(nc, {}, T_FULL)
    r = _CACHE[rkey]
    xt = np.zeros((NCORES * 64, BL * T_FULL), np.float32)
    r.run(xt)
    ws = []
    for _ in range(n_rep):
        t0 = time.time()
        r.run(xt)
        ws.append(time.time() - t0)
    return min(ws)
